# revision 1
# baseline (speedup 1.0000x reference)
"""Trainium2 Bass kernel v4 for nn_ContentLoss (Gaussian-blur content MSE).

Math: MSE( blur61(a).mean(ch), blur61(b).mean(ch) ) with a 61x61 Gaussian
(sigma=1, separable) and reflect padding.  Everything before the final square
is linear, so each core computes g = colconv(rowconv(mean_ch(a - b))) per
image as two banded matmuls on the TensorEngine.

The kernel is HBM-bound (12.6 MB of input per core at ~358 GB/s ~= 35 us),
so the structure minimizes the serial tail after the last input byte lands:

  * float32r matmuls: 1 cycle/row (vs 4 for fp32) at free-size >= 256.  All
    matmul-operand tiles (d, y1, coefficients) are allocated float32r so the
    producing DVE/ACT op rounds on write (BIR verifier requirement).
  * channel combine d = a0+a1+a2-b0-b1-b2 runs incrementally on the DVE as
    each plane's DMA lands; the last two planes' DMAs are split (halves /
    h-chunk quarters) so only ~0.5 us of combine remains after the last byte.
  * pass 1 is emitted chunk-outer so the per-chunk K-accumulation into the
    four PSUM banks starts as soon as that h-chunk of d is combined.
  * y1 PSUM->SBUF copies alternate ACT/DVE so they drain in parallel.
  * one-matmul PE "keep-warm" fillers after most plane DMAs hold the HAM
    clock gate at 2.4 GHz through the DMA phase (PE would otherwise
    re-throttle to 1.2 GHz after ~3.4 us idle).
  * consts shipped fp16 (~0.7 MB instead of 1.7 MB), converted on ACT.

Sharding: pure data parallel, 2 images per core across 8 cores.  Each core
returns per-partition partial sums of g**2; the host reduces in float64.
"""

import sys

import numpy as np

if "/opt/trn_rl_repo" not in sys.path:
    sys.path.insert(0, "/opt/trn_rl_repo")

N = 512
P = 128
IMGS_PER_CORE = 2
N_CORES = 8
FULL_BATCH = 16


def _build_B():
    """B[i, j]: 1D 61-tap normalized Gaussian conv matrix with reflect pad."""
    x = np.arange(61, dtype=np.float64)
    k1 = np.exp(-((x - 30.0) ** 2) / 2.0)
    k1n = k1 / k1.sum()
    B = np.zeros((N, N), np.float64)
    i = np.arange(N)
    for t in range(61):
        j = i + (t - 30)
        j = np.abs(j)
        j = np.where(j > N - 1, 2 * (N - 1) - j, j)
        np.add.at(B, (i, j), k1n[t])
    return B


def _build_consts():
    B = _build_B()
    # pass 1 moving operand: R1[k, c, n] = B[n, 4k+c] / 3
    R1 = np.zeros((P, 4, N), np.float16)
    for c in range(4):
        R1[:, c, :] = (B[:, c::4].T / 3.0).astype(np.float16)
    BT = B.T
    c2main = np.zeros((P, 4, 128), np.float16)
    for m in range(4):
        c2main[:, m, :] = BT[128 * m : 128 * (m + 1), 128 * m : 128 * (m + 1)]
    # off-diagonal blocks are interior Toeplitz (band +-30 never reaches the
    # reflect-corrected borders inside them), so one block serves all m.
    c2prev = BT[0:128, 128:256].astype(np.float16)  # B[128m+j, 128(m-1)+p]
    c2next = BT[256:384, 128:256].astype(np.float16)  # B[128m+j, 128(m+1)+p]
    return R1, c2main, c2prev, c2next


def build_nc():
    from contextlib import ExitStack

    import concourse.bacc as bacc
    import concourse.tile as tile
    from concourse import mybir

    f32 = mybir.dt.float32
    f16 = mybir.dt.float16
    f32r = mybir.dt.float32r
    nc = bacc.Bacc(
        "TRN2", target_bir_lowering=False, debug=False, num_devices=N_CORES
    )

    a = nc.dram_tensor("a", [IMGS_PER_CORE, 3, N, N], f32, kind="ExternalInput")
    b = nc.dram_tensor("b", [IMGS_PER_CORE, 3, N, N], f32, kind="ExternalInput")
    # out[p, 4*img+m] = partial sums of gT[128m+p, :]**2; col 8 is PE-filler
    # garbage (excluded by the host reduction).
    out = nc.dram_tensor(
        "out", [P, 4 * IMGS_PER_CORE + 1], f32, kind="ExternalOutput"
    )

    R1_np, c2main_np, c2prev_np, c2next_np = _build_consts()
    R1_d = nc.inline_tensor(R1_np, name="R1")
    c2main_d = nc.inline_tensor(c2main_np, name="c2main")
    c2prev_d = nc.inline_tensor(c2prev_np, name="c2prev")
    c2next_d = nc.inline_tensor(c2next_np, name="c2next")

    with tile.TileContext(nc) as tc, ExitStack() as ctx:
        consts = ctx.enter_context(tc.tile_pool(name="consts", bufs=1))
        planes = ctx.enter_context(tc.tile_pool(name="planes", bufs=12))
        dpool = ctx.enter_context(tc.tile_pool(name="dpool", bufs=2))
        y1pool = ctx.enter_context(tc.tile_pool(name="y1pool", bufs=2))
        accp = ctx.enter_context(tc.tile_pool(name="accp", bufs=1))
        scratchp = ctx.enter_context(tc.tile_pool(name="scratchp", bufs=2))
        psum1 = ctx.enter_context(tc.tile_pool(name="psum1", bufs=4, space="PSUM"))
        psum2 = ctx.enter_context(tc.tile_pool(name="psum2", bufs=3, space="PSUM"))
        psumw = ctx.enter_context(tc.tile_pool(name="psumw", bufs=1, space="PSUM"))

        # --- consts: fp16 DMA + fp32r conversion on ACT
        r1_h = consts.tile([P, 4, N], f16, name="r1_h")
        nc.sync.dma_start(out=r1_h, in_=R1_d.ap())
        c2main_h = consts.tile([P, 4, 128], f16, name="c2main_h")
        nc.sync.dma_start(out=c2main_h, in_=c2main_d.ap())
        c2prev_h = consts.tile([P, 128], f16, name="c2prev_h")
        nc.sync.dma_start(out=c2prev_h, in_=c2prev_d.ap())
        c2next_h = consts.tile([P, 128], f16, name="c2next_h")
        nc.sync.dma_start(out=c2next_h, in_=c2next_d.ap())

        r1_t = consts.tile([P, 4, N], f32r, name="r1_t")
        nc.scalar.copy(r1_t, r1_h)
        c2main_t = consts.tile([P, 4, 128], f32r, name="c2main_t")
        nc.scalar.copy(c2main_t, c2main_h)
        c2prev_t = consts.tile([P, 128], f32r, name="c2prev_t")
        nc.scalar.copy(c2prev_t, c2prev_h)
        c2next_t = consts.tile([P, 128], f32r, name="c2next_t")
        nc.scalar.copy(c2next_t, c2next_h)

        acc_t = accp.tile([P, 4 * IMGS_PER_CORE + 1], f32, name="acc_t")

        psw = psumw.tile([P, N], f32, name="psw")
        n_fill = [0]

        def filler(lhsT, rhs):
            """Single-matmul keep-warm group into the shared psw bank."""
            nc.tensor.matmul(
                psw[:, 0 : rhs.free_size()],
                lhsT=lhsT,
                rhs=rhs,
                start=True,
                stop=True,
            )
            n_fill[0] += 1

        # hold the clock gate open right after the consts land
        filler(r1_h[:, 0, 0:128], r1_h[:, 1, 0:128])
        filler(r1_h[:, 2, 0:128], r1_h[:, 3, 0:128])

        for img in range(IMGS_PER_CORE):
            # --- load the 6 planes, h-interleaved: [p, c, w] = row 4p+c.
            # b1 is split into chunk-pair halves and b2 into h-chunk quarters
            # so the tail combine/pass-1 can start before the last byte.
            plane_ts = []
            for pi, (src, src_name, ch) in enumerate(
                (s, n, c) for s, n in ((a, "a"), (b, "b")) for c in range(3)
            ):
                pt = planes.tile(
                    [P, 4, N], f32, name=f"pl_{src_name}{img}c{ch}", tag="pl"
                )
                src_ap = src.ap()[img, ch].rearrange("(p c) w -> p c w", p=P)
                if pi == 4:  # b1: two chunk-pair halves
                    nc.sync.dma_start(out=pt[:, 0:2, :], in_=src_ap[:, 0:2, :])
                    nc.sync.dma_start(out=pt[:, 2:4, :], in_=src_ap[:, 2:4, :])
                elif pi == 5:  # b2: four h-chunk quarters
                    for c in range(4):
                        nc.sync.dma_start(
                            out=pt[:, c, :], in_=src_ap[:, c, :]
                        )
                else:
                    nc.sync.dma_start(out=pt, in_=src_ap)
                # keep-warm filler (skip the final two planes: PE has real
                # work by then and fillers would head-of-line block pass 1).
                # 128-row fp32 matmul = ~0.2 us: enough HAM activity, cheap
                # if the scheduler drops one into the critical tail.
                if pi < 4:
                    filler(pt[:, 0, 0:128], pt[:, 1, 0:128])
                plane_ts.append(pt)

            # --- channel combine on DVE, incremental as planes land:
            # d = a0+a1+a2-b0-b1-b2 (the 1/3 lives in R1)
            d = dpool.tile([P, 4, N], f32r, name=f"d_{img}", tag="d")
            nc.vector.tensor_add(d, plane_ts[0], plane_ts[1])
            nc.vector.tensor_add(d, d, plane_ts[2])
            nc.vector.tensor_sub(d, d, plane_ts[3])
            for half in range(2):
                hs = slice(2 * half, 2 * half + 2)
                nc.vector.tensor_sub(
                    d[:, hs, :], d[:, hs, :], plane_ts[4][:, hs, :]
                )
            for c in range(4):
                nc.vector.tensor_sub(
                    d[:, c, :], d[:, c, :], plane_ts[5][:, c, :]
                )

            # --- pass 1, chunk-outer: ps1[wc] += d[:,c,wslice].T @ R1[:,c,:]
            # as each h-chunk of d completes
            ps1 = [
                psum1.tile([P, N], f32, name=f"ps1_{img}_{wc}", tag="ps1")
                for wc in range(4)
            ]
            for c in range(4):
                for wc in range(4):
                    nc.tensor.matmul(
                        ps1[wc],
                        lhsT=d[:, c, 128 * wc : 128 * (wc + 1)],
                        rhs=r1_t[:, c, :],
                        start=(c == 0),
                        stop=(c == 3),
                    )
            y1 = y1pool.tile([P, 4, N], f32r, name=f"y1_{img}", tag="y1")
            for wc in range(4):
                if wc % 2 == 0:
                    nc.scalar.copy(y1[:, wc, :], ps1[wc])
                else:
                    nc.vector.tensor_copy(y1[:, wc, :], ps1[wc])

            # --- pass 2: gT chunk per w_out chunk m, then square+row-reduce.
            # Off-diagonal blocks first (start=True marks the bank); the
            # full-bank main block closes every byte's group with stop=True.
            for m in range(4):
                ps2 = psum2.tile([P, N], f32, name=f"ps2_{img}_{m}", tag="ps2")
                first = True
                if m > 0:
                    nc.tensor.matmul(
                        ps2,
                        lhsT=c2prev_t,
                        rhs=y1[:, m - 1, :],
                        start=first,
                        stop=False,
                    )
                    first = False
                if m < 3:
                    nc.tensor.matmul(
                        ps2,
                        lhsT=c2next_t,
                        rhs=y1[:, m + 1, :],
                        start=first,
                        stop=False,
                    )
                    first = False
                nc.tensor.matmul(
                    ps2,
                    lhsT=c2main_t[:, m, :],
                    rhs=y1[:, m, :],
                    start=False,
                    stop=True,
                )
                scr = scratchp.tile([P, N], f32, name=f"scr_{img}_{m}", tag="scr")
                nc.scalar.activation(
                    scr,
                    ps2,
                    mybir.ActivationFunctionType.Square,
                    accum_out=acc_t[:, 4 * img + m : 4 * img + m + 1],
                )

        # consume the filler bank so the keep-warm matmuls stay live
        scrw = scratchp.tile([P, N], f32, name="scrw", tag="scr")
        nc.scalar.activation(
            scrw[:, 0:128],
            psw[:, 0:128],
            mybir.ActivationFunctionType.Square,
            accum_out=acc_t[:, 8:9],
        )

        nc.sync.dma_start(out=out.ap(), in_=acc_t)

    nc.finalize()
    return nc


_CACHE = {}


def _get_nc():
    if "nc" not in _CACHE:
        _CACHE["nc"] = build_nc()
    return _CACHE["nc"]


def run(inputs, **spmd_kwargs):
    """Run on 8 cores; returns (scalar_result, BassKernelResults)."""
    from concourse import bass_utils

    a = np.ascontiguousarray(np.asarray(inputs["a"], dtype=np.float32))
    b = np.ascontiguousarray(np.asarray(inputs["b"], dtype=np.float32))
    assert a.shape == (FULL_BATCH, 3, N, N) and b.shape == a.shape

    nc = _get_nc()
    in_maps = []
    for core in range(N_CORES):
        sl = slice(core * IMGS_PER_CORE, (core + 1) * IMGS_PER_CORE)
        in_maps.append(
            {
                "a": np.ascontiguousarray(a[sl]),
                "b": np.ascontiguousarray(b[sl]),
            }
        )
    res = bass_utils.run_bass_kernel_spmd(
        nc, in_maps, core_ids=list(range(N_CORES)), **spmd_kwargs
    )
    total = 0.0
    for r in res.results:
        total += np.asarray(r["out"])[:, :8].astype(np.float64).sum()
    mse = np.float32(total / (FULL_BATCH * N * N))
    return np.asarray(mse, dtype=np.float32), res


# ---------------------------------------------------------------------------
# Fallback: the previous known-good kernel (fp32 matmuls, windowed pass 1).
# Used only if the primary path fails to compile/run for any reason.



N = 512
P = 128
S = 16  # pass-1 h_out window halo
IMGS_PER_CORE = 2
N_CORES = 8
FULL_BATCH = 16


def _bl_bl_build_B():
    """B[i, j]: 1D 61-tap normalized Gaussian conv matrix with reflect pad."""
    x = np.arange(61, dtype=np.float64)
    k1 = np.exp(-((x - 30.0) ** 2) / 2.0)
    k1n = k1 / k1.sum()
    B = np.zeros((N, N), np.float64)
    i = np.arange(N)
    for t in range(61):
        j = i + (t - 30)
        j = np.abs(j)
        j = np.where(j > N - 1, 2 * (N - 1) - j, j)
        np.add.at(B, (i, j), k1n[t])
    return B


def _bl_bl_windows():
    return [
        (max(0, 128 * c - S), min(N, 128 * c + 128 + S)) for c in range(4)
    ]


def _bl_bl_build_consts():
    BT = _bl_build_B().T.copy()
    W = _bl_windows()
    wmax = max(hi - lo for lo, hi in W)
    c1p = np.zeros((P, 4, wmax), np.float32)
    for c, (lo, hi) in enumerate(W):
        c1p[:, c, : hi - lo] = (BT[128 * c : 128 * (c + 1), lo:hi] / 3.0)
    c1m = -c1p
    c2main = np.zeros((P, 4, 128), np.float32)
    for m in range(4):
        c2main[:, m, :] = BT[128 * m : 128 * (m + 1), 128 * m : 128 * (m + 1)]
    # full off-diagonal blocks: BT[chunk m-1, chunk m] and BT[chunk m+1, chunk m]
    # (only ~30 rows near the boundary are nonzero; K=128 costs the same as
    # K=32 on the PE, and full blocks avoid partition-offset operands)
    c2prev = np.zeros((P, 3, 128), np.float32)
    for m in range(1, 4):
        c2prev[:, m - 1, :] = BT[
            128 * (m - 1) : 128 * m, 128 * m : 128 * (m + 1)
        ]
    c2next = np.zeros((P, 3, 128), np.float32)
    for m in range(0, 3):
        c2next[:, m, :] = BT[
            128 * (m + 1) : 128 * (m + 2), 128 * m : 128 * (m + 1)
        ]
    return c1p, c1m, c2main, c2prev, c2next


def _bl_build_nc(const_inline=True, slivers=True):
    from contextlib import ExitStack

    import concourse.bacc as bacc
    import concourse.tile as tile
    from concourse import mybir

    f32 = mybir.dt.float32
    nc = bacc.Bacc(
        "TRN2", target_bir_lowering=False, debug=False, num_devices=N_CORES
    )

    a = nc.dram_tensor("a", [IMGS_PER_CORE, 3, N, N], f32, kind="ExternalInput")
    b = nc.dram_tensor("b", [IMGS_PER_CORE, 3, N, N], f32, kind="ExternalInput")
    # out[p, 4*img+m] = partial sum over (h) of gT[m*128+p-chunk, :]**2
    out = nc.dram_tensor(
        "out", [P, 4 * IMGS_PER_CORE], f32, kind="ExternalOutput"
    )

    c1p_np, c1m_np, c2main_np, c2prev_np, c2next_np = _bl_build_consts()
    if const_inline:
        c1p_d = nc.inline_tensor(c1p_np, name="c1p")
        c1m_d = nc.inline_tensor(c1m_np, name="c1m")
        c2main_d = nc.inline_tensor(c2main_np, name="c2main")
        c2prev_d = nc.inline_tensor(c2prev_np, name="c2prev")
        c2next_d = nc.inline_tensor(c2next_np, name="c2next")
    else:
        c1p_d = nc.dram_tensor("c1p", list(c1p_np.shape), f32, kind="ExternalInput")
        c1m_d = nc.dram_tensor("c1m", list(c1m_np.shape), f32, kind="ExternalInput")
        c2main_d = nc.dram_tensor(
            "c2main", list(c2main_np.shape), f32, kind="ExternalInput"
        )
        c2prev_d = nc.dram_tensor(
            "c2prev", list(c2prev_np.shape), f32, kind="ExternalInput"
        )
        c2next_d = nc.dram_tensor(
            "c2next", list(c2next_np.shape), f32, kind="ExternalInput"
        )

    W = _bl_windows()
    wmax = c1p_np.shape[2]

    with tile.TileContext(nc) as tc, ExitStack() as ctx:
        consts = ctx.enter_context(tc.tile_pool(name="consts", bufs=1))
        planes = ctx.enter_context(tc.tile_pool(name="planes", bufs=12))
        y1pool = ctx.enter_context(tc.tile_pool(name="y1pool", bufs=8))
        accp = ctx.enter_context(tc.tile_pool(name="accp", bufs=1))
        scratchp = ctx.enter_context(tc.tile_pool(name="scratchp", bufs=2))
        psum1 = ctx.enter_context(tc.tile_pool(name="psum1", bufs=4, space="PSUM"))
        psum2 = ctx.enter_context(tc.tile_pool(name="psum2", bufs=4, space="PSUM"))

        c1p_t = consts.tile([P, 4, wmax], f32, name="c1p_t")
        nc.sync.dma_start(out=c1p_t, in_=c1p_d.ap())
        c1m_t = consts.tile([P, 4, wmax], f32, name="c1m_t")
        nc.sync.dma_start(out=c1m_t, in_=c1m_d.ap())
        c2main_t = consts.tile([P, 4, 128], f32, name="c2main_t")
        nc.sync.dma_start(out=c2main_t, in_=c2main_d.ap())
        c2prev_t = consts.tile([P, 3, 128], f32, name="c2prev_t")
        nc.sync.dma_start(out=c2prev_t, in_=c2prev_d.ap())
        c2next_t = consts.tile([P, 3, 128], f32, name="c2next_t")
        nc.sync.dma_start(out=c2next_t, in_=c2next_d.ap())

        acc_t = accp.tile([P, 4 * IMGS_PER_CORE], f32, name="acc_t")

        for img in range(IMGS_PER_CORE):
            # --- load the 6 planes of this image (a ch0..2, then b ch0..2)
            plane_ts = []
            for src, src_name in ((a, "a"), (b, "b")):
                for ch in range(3):
                    pt = planes.tile(
                        [P, 4, N], f32, name=f"pl_{src_name}{img}c{ch}", tag="pl"
                    )
                    nc.sync.dma_start(
                        out=pt,
                        in_=src.ap()[img, ch].rearrange("(c p) w -> p c w", p=P),
                    )
                    plane_ts.append(pt)

            # --- pass 1: y1T[w, h_out] per w-chunk, channel-combine in PSUM
            ps1 = [
                psum1.tile([P, N], f32, name=f"ps1_{img}_{wc}", tag="ps1")
                for wc in range(4)
            ]
            # plane 0 writes the bank in disjoint segments (start=True marks the
            # whole 2KB zero-region pending; each matmul must touch uniformly
            # pending or uniformly written bytes), later planes accumulate.
            for pi in range(6):
                coef_t = c1p_t if pi < 3 else c1m_t
                for wc in range(4):
                    for c in range(4):
                        lo, hi = W[c]
                        if pi == 0:
                            # fresh segment ([lo,hi) minus the 32-wide strip
                            # already written by chunk c-1), then the overlap
                            # strip accumulated separately
                            fresh_lo = lo if c == 0 else 128 * c + S
                            segs = [(fresh_lo, hi, c == 0)]
                            if c > 0:
                                segs.append((128 * c - S, 128 * c + S, False))
                        else:
                            segs = [(lo, hi, False)]
                        for seg_lo, seg_hi, is_start in segs:
                            nc.tensor.matmul(
                                ps1[wc][:, seg_lo:seg_hi],
                                lhsT=plane_ts[pi][:, c, 128 * wc : 128 * (wc + 1)],
                                rhs=coef_t[:, c, seg_lo - lo : seg_hi - lo],
                                start=is_start,
                                stop=(pi == 5 and c == 3),
                            )

            y1 = []
            for wc in range(4):
                yt = y1pool.tile([P, N], f32, name=f"y1_{img}_{wc}", tag="y1")
                nc.scalar.copy(yt, ps1[wc])
                y1.append(yt)

            # --- pass 2: gT chunk per w_out chunk m, then square+row-reduce
            for m in range(4):
                ps2 = psum2.tile([P, N], f32, name=f"ps2_{img}_{m}", tag="ps2")
                n_mm = 1 + (slivers and m > 0) + (slivers and m < 3)
                k = 0
                nc.tensor.matmul(
                    ps2,
                    lhsT=c2main_t[:, m, :],
                    rhs=y1[m],
                    start=True,
                    stop=(k := k + 1) == n_mm,
                )
                if slivers and m > 0:
                    nc.tensor.matmul(
                        ps2,
                        lhsT=c2prev_t[:, m - 1, :],
                        rhs=y1[m - 1],
                        start=False,
                        stop=(k := k + 1) == n_mm,
                    )
                if slivers and m < 3:
                    nc.tensor.matmul(
                        ps2,
                        lhsT=c2next_t[:, m, :],
                        rhs=y1[m + 1],
                        start=False,
                        stop=(k := k + 1) == n_mm,
                    )
                scr = scratchp.tile([P, N], f32, name=f"scr_{img}_{m}", tag="scr")
                nc.scalar.activation(
                    scr,
                    ps2,
                    mybir.ActivationFunctionType.Square,
                    accum_out=acc_t[:, 4 * img + m : 4 * img + m + 1],
                )

        nc.sync.dma_start(out=out.ap(), in_=acc_t)

    nc.finalize()
    return nc


_BL_CACHE = {}


def _bl_get_nc(**opts):
    key = tuple(sorted(opts.items()))
    if key not in _CACHE:
        _BL_CACHE[key] = _bl_build_nc(**opts)
    return _BL_CACHE[key]


def _bl_run(inputs, const_inline=True, slivers=True, **spmd_kwargs):
    """Run on 8 cores; returns (scalar_result, BassKernelResults)."""
    from concourse import bass_utils

    a = np.ascontiguousarray(np.asarray(inputs["a"], dtype=np.float32))
    b = np.ascontiguousarray(np.asarray(inputs["b"], dtype=np.float32))
    assert a.shape == (FULL_BATCH, 3, N, N) and b.shape == a.shape

    nc = _bl_get_nc(const_inline=const_inline, slivers=slivers)
    const_map = {}
    if not const_inline:
        names = ["c1p", "c1m", "c2main", "c2prev", "c2next"]
        const_map = dict(zip(names, _bl_build_consts()))
    in_maps = []
    for core in range(N_CORES):
        sl = slice(core * IMGS_PER_CORE, (core + 1) * IMGS_PER_CORE)
        in_maps.append(
            {
                "a": np.ascontiguousarray(a[sl]),
                "b": np.ascontiguousarray(b[sl]),
                **const_map,
            }
        )
    res = bass_utils.run_bass_kernel_spmd(
        nc, in_maps, core_ids=list(range(N_CORES)), **spmd_kwargs
    )
    total = 0.0
    for r in res.results:
        total += np.asarray(r["out"]).astype(np.float64).sum()
    mse = np.float32(total / (FULL_BATCH * N * N))
    return np.asarray(mse, dtype=np.float32), res




def kernel(**inputs) -> np.ndarray:
    try:
        result, _ = run(inputs)
        return result
    except Exception:
        import traceback

        traceback.print_exc()
        result, _ = _bl_run(inputs)
        return result



# revision 3
# speedup vs baseline: 1.0857x; 1.0857x over previous
"""Trainium2 Bass kernel v5 for nn_ContentLoss (Gaussian-blur content MSE).

Math: MSE( blur61(a).mean(ch), blur61(b).mean(ch) ) with a 61x61 Gaussian
(sigma=1, separable) and reflect padding.  Everything before the final square
is linear, so each core computes g = colconv(rowconv(mean_ch(a - b))) per
image as two banded matmuls on the TensorEngine.

The kernel is DMA-bound (12.58 MB of fp32 input per core at ~425 GB/s
sustained ~= 30 us), so v5 minimizes everything off that critical path:

  * all 12 plane loads go through SWDGE (gpsimd) DMA with an inline
    fp32->bf16 cast: the HBM read side still runs at full rate (measured
    ~426 GB/s) while every downstream consumer gets 2x cheaper data.
  * channel combine d = a0+a1+a2-b0-b1-b2 runs on the DVE in bf16 2x mode
    (1.07 us per full-plane op vs 2.3 us for fp32), incrementally as planes
    land.  DVE total load drops from ~38 us (v4 bottleneck) to ~15 us.
  * all matmuls are bf16 (1 cycle/row at any free size + fast weight load).
  * consts ship as ONE packed bf16 inline tensor on the HWDGE queue
    (parallel to the SWDGE plane stream) and are used directly -- no
    conversion pass.
  * the last plane of the last image is split (c-quarters, then the final
    quarter into w-halves) so the post-last-byte tail is just: one
    [128,256] sub, 2 stop-matmuls, y1 copies (split ACT/DVE), 8 pass-2
    matmuls and the final squares (alternating ACT / DVE
    tensor_tensor_reduce).
  * one-matmul PE keep-warm fillers after plane DMAs hold the HAM clock
    gate at 2.4 GHz through the DMA phase.

Sharding: pure data parallel, 2 images per core across 8 cores.  Each core
returns per-partition partial sums of g**2; the host reduces in float64.
"""

import sys

import numpy as np

if "/opt/trn_rl_repo" not in sys.path:
    sys.path.insert(0, "/opt/trn_rl_repo")

N = 512
P = 128
IMGS_PER_CORE = 2
N_CORES = 8
FULL_BATCH = 16


def _build_B():
    """B[i, j]: 1D 61-tap normalized Gaussian conv matrix with reflect pad."""
    x = np.arange(61, dtype=np.float64)
    k1 = np.exp(-((x - 30.0) ** 2) / 2.0)
    k1n = k1 / k1.sum()
    B = np.zeros((N, N), np.float64)
    i = np.arange(N)
    for t in range(61):
        j = i + (t - 30)
        j = np.abs(j)
        j = np.where(j > N - 1, 2 * (N - 1) - j, j)
        np.add.at(B, (i, j), k1n[t])
    return B


def _build_consts_v5():
    """One packed [128, 2816] bf16 matrix: r1 | c2main | c2prev | c2next.

    r1   [:, c*512+n]    = B[n, 4k+c] / 3      (pass-1 rhs, per c-chunk)
    c2m  [:, 2048+128m+j] = BT[128m+k, 128m+j] (pass-2 diag block, per m)
    c2p  [:, 2560+j]      = BT[k, 128+j]       (pass-2 off-diag, interior
    c2n  [:, 2688+j]      = BT[256+k, 128+j]    Toeplitz: one block fits all m)
    """
    import ml_dtypes

    B = _build_B()
    BT = B.T
    pack = np.zeros((P, 2816), np.float64)
    for c in range(4):
        pack[:, 512 * c : 512 * (c + 1)] = B[:, c::4].T / 3.0
    for m in range(4):
        pack[:, 2048 + 128 * m : 2048 + 128 * (m + 1)] = BT[
            128 * m : 128 * (m + 1), 128 * m : 128 * (m + 1)
        ]
    pack[:, 2560:2688] = BT[0:128, 128:256]
    pack[:, 2688:2816] = BT[256:384, 128:256]
    return pack.astype(ml_dtypes.bfloat16)


def build_nc():
    from contextlib import ExitStack

    import concourse.bacc as bacc
    import concourse.tile as tile
    from concourse import mybir

    f32 = mybir.dt.float32
    bf16 = mybir.dt.bfloat16
    nc = bacc.Bacc(
        "TRN2", target_bir_lowering=False, debug=False, num_devices=N_CORES
    )

    a = nc.dram_tensor("a", [IMGS_PER_CORE, 3, N, N], f32, kind="ExternalInput")
    b = nc.dram_tensor("b", [IMGS_PER_CORE, 3, N, N], f32, kind="ExternalInput")
    # out[p, 4*img+m] = partial sums of gT[128m+p, :]**2; col 8 is PE-filler
    # garbage (excluded by the host reduction).
    out = nc.dram_tensor(
        "out", [P, 4 * IMGS_PER_CORE + 1], f32, kind="ExternalOutput"
    )

    consts_d = nc.inline_tensor(_build_consts_v5(), name="constpack")

    with tile.TileContext(nc) as tc, ExitStack() as ctx:
        consts = ctx.enter_context(tc.tile_pool(name="consts", bufs=1))
        planes = ctx.enter_context(tc.tile_pool(name="planes", bufs=12))
        dpool = ctx.enter_context(tc.tile_pool(name="dpool", bufs=2))
        y1pool = ctx.enter_context(tc.tile_pool(name="y1pool", bufs=2))
        accp = ctx.enter_context(tc.tile_pool(name="accp", bufs=1))
        scratchp = ctx.enter_context(tc.tile_pool(name="scratchp", bufs=2))
        psum1 = ctx.enter_context(tc.tile_pool(name="psum1", bufs=4, space="PSUM"))
        psum2 = ctx.enter_context(tc.tile_pool(name="psum2", bufs=3, space="PSUM"))
        psumw = ctx.enter_context(tc.tile_pool(name="psumw", bufs=1, space="PSUM"))

        # --- consts: one bf16 DMA on the HWDGE (sync) queue, used directly
        ct = consts.tile([P, 2816], bf16, name="ct")
        nc.sync.dma_start(out=ct, in_=consts_d.ap())

        def r1(c):
            return ct[:, 512 * c : 512 * (c + 1)]

        def c2main(m):
            return ct[:, 2048 + 128 * m : 2048 + 128 * (m + 1)]

        c2prev = ct[:, 2560:2688]
        c2next = ct[:, 2688:2816]

        acc_t = accp.tile([P, 4 * IMGS_PER_CORE + 1], f32, name="acc_t")

        psw = psumw.tile([P, N], f32, name="psw")

        def filler(lhsT, rhs):
            """Single-matmul keep-warm group into the shared psw bank."""
            nc.tensor.matmul(
                psw[:, 0 : rhs.free_size()],
                lhsT=lhsT,
                rhs=rhs,
                start=True,
                stop=True,
            )

        # hold the clock gate open right after the consts land
        filler(ct[:, 0:128], ct[:, 128:256])
        filler(ct[:, 256:384], ct[:, 384:512])

        for img in range(IMGS_PER_CORE):
            last = img == IMGS_PER_CORE - 1
            # --- load the 6 planes via SWDGE with fp32->bf16 cast.
            # h-interleaved: [p, c, w] = row 4p+c.  b1 is split into
            # chunk-pair halves; b2 into c-quarters (last image: the final
            # quarter further into w-halves) so the tail combine/pass-1
            # starts before the last byte.
            plane_ts = []
            for pi, (src, src_name, ch) in enumerate(
                (s, n_, c) for s, n_ in ((a, "a"), (b, "b")) for c in range(3)
            ):
                pt = planes.tile(
                    [P, 4, N], bf16, name=f"pl_{src_name}{img}c{ch}", tag="pl"
                )
                src_ap = src.ap()[img, ch].rearrange("(p c) w -> p c w", p=P)
                if pi == 4:  # b1: two chunk-pair halves
                    nc.gpsimd.dma_start(out=pt[:, 0:2, :], in_=src_ap[:, 0:2, :])
                    nc.gpsimd.dma_start(out=pt[:, 2:4, :], in_=src_ap[:, 2:4, :])
                elif pi == 5:  # b2: c-quarters (+ w-half split on last img)
                    for c in range(4):
                        if last and c == 3:
                            for wh in range(2):
                                ws = slice(256 * wh, 256 * (wh + 1))
                                nc.gpsimd.dma_start(
                                    out=pt[:, c, ws], in_=src_ap[:, c, ws]
                                )
                        else:
                            nc.gpsimd.dma_start(
                                out=pt[:, c, :], in_=src_ap[:, c, :]
                            )
                else:
                    nc.gpsimd.dma_start(out=pt, in_=src_ap)
                # keep-warm filler (~0.1 us bf16): enough HAM activity,
                # cheap if the scheduler drops one into the critical tail.
                if pi < 5:
                    filler(pt[:, 0, 0:128], pt[:, 1, 0:128])
                plane_ts.append(pt)

            # --- channel combine on DVE (bf16 2x mode), incremental as
            # planes land: d = a0+a1+a2-b0-b1-b2 (the 1/3 lives in r1)
            d = dpool.tile([P, 4, N], bf16, name=f"d_{img}", tag="d")
            nc.vector.tensor_add(d, plane_ts[0], plane_ts[1])
            nc.vector.tensor_add(d, d, plane_ts[2])
            nc.vector.tensor_sub(d, d, plane_ts[3])
            for half in range(2):
                hs = slice(2 * half, 2 * half + 2)
                nc.vector.tensor_sub(
                    d[:, hs, :], d[:, hs, :], plane_ts[4][:, hs, :]
                )
            for c in range(4):
                if last and c == 3:
                    for wh in range(2):
                        ws = slice(256 * wh, 256 * (wh + 1))
                        nc.vector.tensor_sub(
                            d[:, c, ws], d[:, c, ws], plane_ts[5][:, c, ws]
                        )
                else:
                    nc.vector.tensor_sub(
                        d[:, c, :], d[:, c, :], plane_ts[5][:, c, :]
                    )

            # --- pass 1: ps1[wc] += d[:,c,wslice].T @ r1[c], K-accumulated
            # over c so each bank's stop-matmul fires as its last d-chunk
            # completes.
            ps1 = [
                psum1.tile([P, N], f32, name=f"ps1_{img}_{wc}", tag="ps1")
                for wc in range(4)
            ]
            for c in range(4):
                for wc in range(4):
                    nc.tensor.matmul(
                        ps1[wc],
                        lhsT=d[:, c, 128 * wc : 128 * (wc + 1)],
                        rhs=r1(c),
                        start=(c == 0),
                        stop=(c == 3),
                    )
            # y1 copies: each wc split into halves across ACT and DVE so the
            # tail pair drains in parallel.
            y1 = y1pool.tile([P, 4, N], bf16, name=f"y1_{img}", tag="y1")
            for wc in range(4):
                nc.scalar.copy(y1[:, wc, 0:256], ps1[wc][:, 0:256])
                nc.vector.tensor_copy(y1[:, wc, 256:512], ps1[wc][:, 256:512])

            # --- pass 2: gT chunk per w_out chunk m, then square+row-reduce
            # (alternating ACT activation / DVE tensor_tensor_reduce).
            for m in range(4):
                ps2 = psum2.tile([P, N], f32, name=f"ps2_{img}_{m}", tag="ps2")
                first = True
                if m > 0:
                    nc.tensor.matmul(
                        ps2,
                        lhsT=c2prev,
                        rhs=y1[:, m - 1, :],
                        start=first,
                        stop=False,
                    )
                    first = False
                if m < 3:
                    nc.tensor.matmul(
                        ps2,
                        lhsT=c2next,
                        rhs=y1[:, m + 1, :],
                        start=first,
                        stop=False,
                    )
                    first = False
                nc.tensor.matmul(
                    ps2,
                    lhsT=c2main(m),
                    rhs=y1[:, m, :],
                    start=False,
                    stop=True,
                )
                col = acc_t[:, 4 * img + m : 4 * img + m + 1]
                scr = scratchp.tile([P, N], f32, name=f"scr_{img}_{m}", tag="scr")
                nc.scalar.activation(
                    scr,
                    ps2,
                    mybir.ActivationFunctionType.Square,
                    accum_out=col,
                )

        # consume the filler bank so the keep-warm matmuls stay live
        scrw = scratchp.tile([P, N], f32, name="scrw", tag="scr")
        nc.scalar.activation(
            scrw[:, 0:128],
            psw[:, 0:128],
            mybir.ActivationFunctionType.Square,
            accum_out=acc_t[:, 8:9],
        )

        nc.sync.dma_start(out=out.ap(), in_=acc_t)

    nc.finalize()
    return nc


_CACHE = {}


def _get_nc():
    if "nc" not in _CACHE:
        _CACHE["nc"] = build_nc()
    return _CACHE["nc"]


def run(inputs, **spmd_kwargs):
    """Run on 8 cores; returns (scalar_result, BassKernelResults)."""
    from concourse import bass_utils

    a = np.ascontiguousarray(np.asarray(inputs["a"], dtype=np.float32))
    b = np.ascontiguousarray(np.asarray(inputs["b"], dtype=np.float32))
    assert a.shape == (FULL_BATCH, 3, N, N) and b.shape == a.shape

    nc = _get_nc()
    in_maps = []
    for core in range(N_CORES):
        sl = slice(core * IMGS_PER_CORE, (core + 1) * IMGS_PER_CORE)
        in_maps.append(
            {
                "a": np.ascontiguousarray(a[sl]),
                "b": np.ascontiguousarray(b[sl]),
            }
        )
    res = bass_utils.run_bass_kernel_spmd(
        nc, in_maps, core_ids=list(range(N_CORES)), **spmd_kwargs
    )
    total = 0.0
    for r in res.results:
        total += np.asarray(r["out"])[:, :8].astype(np.float64).sum()
    mse = np.float32(total / (FULL_BATCH * N * N))
    return np.asarray(mse, dtype=np.float32), res


# ---------------------------------------------------------------------------
# Fallback: kernel v4 (fp32 DVE combine + f32r matmuls) — known-good on HW.
# Used only if the primary path fails to compile/run for any reason.


def _v4_build_consts():
    B = _build_B()
    R1 = np.zeros((P, 4, N), np.float16)
    for c in range(4):
        R1[:, c, :] = (B[:, c::4].T / 3.0).astype(np.float16)
    BT = B.T
    c2main = np.zeros((P, 4, 128), np.float16)
    for m in range(4):
        c2main[:, m, :] = BT[128 * m : 128 * (m + 1), 128 * m : 128 * (m + 1)]
    c2prev = BT[0:128, 128:256].astype(np.float16)
    c2next = BT[256:384, 128:256].astype(np.float16)
    return R1, c2main, c2prev, c2next


def _v4_build_nc():
    from contextlib import ExitStack

    import concourse.bacc as bacc
    import concourse.tile as tile
    from concourse import mybir

    f32 = mybir.dt.float32
    f16 = mybir.dt.float16
    f32r = mybir.dt.float32r
    nc = bacc.Bacc(
        "TRN2", target_bir_lowering=False, debug=False, num_devices=N_CORES
    )

    a = nc.dram_tensor("a", [IMGS_PER_CORE, 3, N, N], f32, kind="ExternalInput")
    b = nc.dram_tensor("b", [IMGS_PER_CORE, 3, N, N], f32, kind="ExternalInput")
    out = nc.dram_tensor(
        "out", [P, 4 * IMGS_PER_CORE + 1], f32, kind="ExternalOutput"
    )

    R1_np, c2main_np, c2prev_np, c2next_np = _v4_build_consts()
    R1_d = nc.inline_tensor(R1_np, name="R1")
    c2main_d = nc.inline_tensor(c2main_np, name="c2main")
    c2prev_d = nc.inline_tensor(c2prev_np, name="c2prev")
    c2next_d = nc.inline_tensor(c2next_np, name="c2next")

    with tile.TileContext(nc) as tc, ExitStack() as ctx:
        consts = ctx.enter_context(tc.tile_pool(name="consts", bufs=1))
        planes = ctx.enter_context(tc.tile_pool(name="planes", bufs=12))
        dpool = ctx.enter_context(tc.tile_pool(name="dpool", bufs=2))
        y1pool = ctx.enter_context(tc.tile_pool(name="y1pool", bufs=2))
        accp = ctx.enter_context(tc.tile_pool(name="accp", bufs=1))
        scratchp = ctx.enter_context(tc.tile_pool(name="scratchp", bufs=2))
        psum1 = ctx.enter_context(tc.tile_pool(name="psum1", bufs=4, space="PSUM"))
        psum2 = ctx.enter_context(tc.tile_pool(name="psum2", bufs=3, space="PSUM"))
        psumw = ctx.enter_context(tc.tile_pool(name="psumw", bufs=1, space="PSUM"))

        r1_h = consts.tile([P, 4, N], f16, name="r1_h")
        nc.sync.dma_start(out=r1_h, in_=R1_d.ap())
        c2main_h = consts.tile([P, 4, 128], f16, name="c2main_h")
        nc.sync.dma_start(out=c2main_h, in_=c2main_d.ap())
        c2prev_h = consts.tile([P, 128], f16, name="c2prev_h")
        nc.sync.dma_start(out=c2prev_h, in_=c2prev_d.ap())
        c2next_h = consts.tile([P, 128], f16, name="c2next_h")
        nc.sync.dma_start(out=c2next_h, in_=c2next_d.ap())

        r1_t = consts.tile([P, 4, N], f32r, name="r1_t")
        nc.scalar.copy(r1_t, r1_h)
        c2main_t = consts.tile([P, 4, 128], f32r, name="c2main_t")
        nc.scalar.copy(c2main_t, c2main_h)
        c2prev_t = consts.tile([P, 128], f32r, name="c2prev_t")
        nc.scalar.copy(c2prev_t, c2prev_h)
        c2next_t = consts.tile([P, 128], f32r, name="c2next_t")
        nc.scalar.copy(c2next_t, c2next_h)

        acc_t = accp.tile([P, 4 * IMGS_PER_CORE + 1], f32, name="acc_t")

        psw = psumw.tile([P, N], f32, name="psw")

        def filler(lhsT, rhs):
            nc.tensor.matmul(
                psw[:, 0 : rhs.free_size()],
                lhsT=lhsT,
                rhs=rhs,
                start=True,
                stop=True,
            )

        filler(r1_h[:, 0, 0:128], r1_h[:, 1, 0:128])
        filler(r1_h[:, 2, 0:128], r1_h[:, 3, 0:128])

        for img in range(IMGS_PER_CORE):
            plane_ts = []
            for pi, (src, src_name, ch) in enumerate(
                (s, n_, c) for s, n_ in ((a, "a"), (b, "b")) for c in range(3)
            ):
                pt = planes.tile(
                    [P, 4, N], f32, name=f"pl_{src_name}{img}c{ch}", tag="pl"
                )
                src_ap = src.ap()[img, ch].rearrange("(p c) w -> p c w", p=P)
                if pi == 4:
                    nc.sync.dma_start(out=pt[:, 0:2, :], in_=src_ap[:, 0:2, :])
                    nc.sync.dma_start(out=pt[:, 2:4, :], in_=src_ap[:, 2:4, :])
                elif pi == 5:
                    for c in range(4):
                        nc.sync.dma_start(
                            out=pt[:, c, :], in_=src_ap[:, c, :]
                        )
                else:
                    nc.sync.dma_start(out=pt, in_=src_ap)
                if pi < 4:
                    filler(pt[:, 0, 0:128], pt[:, 1, 0:128])
                plane_ts.append(pt)

            d = dpool.tile([P, 4, N], f32r, name=f"d_{img}", tag="d")
            nc.vector.tensor_add(d, plane_ts[0], plane_ts[1])
            nc.vector.tensor_add(d, d, plane_ts[2])
            nc.vector.tensor_sub(d, d, plane_ts[3])
            for half in range(2):
                hs = slice(2 * half, 2 * half + 2)
                nc.vector.tensor_sub(
                    d[:, hs, :], d[:, hs, :], plane_ts[4][:, hs, :]
                )
            for c in range(4):
                nc.vector.tensor_sub(
                    d[:, c, :], d[:, c, :], plane_ts[5][:, c, :]
                )

            ps1 = [
                psum1.tile([P, N], f32, name=f"ps1_{img}_{wc}", tag="ps1")
                for wc in range(4)
            ]
            for c in range(4):
                for wc in range(4):
                    nc.tensor.matmul(
                        ps1[wc],
                        lhsT=d[:, c, 128 * wc : 128 * (wc + 1)],
                        rhs=r1_t[:, c, :],
                        start=(c == 0),
                        stop=(c == 3),
                    )
            y1 = y1pool.tile([P, 4, N], f32r, name=f"y1_{img}", tag="y1")
            for wc in range(4):
                if wc % 2 == 0:
                    nc.scalar.copy(y1[:, wc, :], ps1[wc])
                else:
                    nc.vector.tensor_copy(y1[:, wc, :], ps1[wc])

            for m in range(4):
                ps2 = psum2.tile([P, N], f32, name=f"ps2_{img}_{m}", tag="ps2")
                first = True
                if m > 0:
                    nc.tensor.matmul(
                        ps2,
                        lhsT=c2prev_t,
                        rhs=y1[:, m - 1, :],
                        start=first,
                        stop=False,
                    )
                    first = False
                if m < 3:
                    nc.tensor.matmul(
                        ps2,
                        lhsT=c2next_t,
                        rhs=y1[:, m + 1, :],
                        start=first,
                        stop=False,
                    )
                    first = False
                nc.tensor.matmul(
                    ps2,
                    lhsT=c2main_t[:, m, :],
                    rhs=y1[:, m, :],
                    start=False,
                    stop=True,
                )
                scr = scratchp.tile([P, N], f32, name=f"scr_{img}_{m}", tag="scr")
                nc.scalar.activation(
                    scr,
                    ps2,
                    mybir.ActivationFunctionType.Square,
                    accum_out=acc_t[:, 4 * img + m : 4 * img + m + 1],
                )

        scrw = scratchp.tile([P, N], f32, name="scrw", tag="scr")
        nc.scalar.activation(
            scrw[:, 0:128],
            psw[:, 0:128],
            mybir.ActivationFunctionType.Square,
            accum_out=acc_t[:, 8:9],
        )

        nc.sync.dma_start(out=out.ap(), in_=acc_t)

    nc.finalize()
    return nc


def _v4_run(inputs, **spmd_kwargs):
    from concourse import bass_utils

    a = np.ascontiguousarray(np.asarray(inputs["a"], dtype=np.float32))
    b = np.ascontiguousarray(np.asarray(inputs["b"], dtype=np.float32))
    assert a.shape == (FULL_BATCH, 3, N, N) and b.shape == a.shape

    if "v4" not in _CACHE:
        _CACHE["v4"] = _v4_build_nc()
    nc = _CACHE["v4"]
    in_maps = []
    for core in range(N_CORES):
        sl = slice(core * IMGS_PER_CORE, (core + 1) * IMGS_PER_CORE)
        in_maps.append(
            {
                "a": np.ascontiguousarray(a[sl]),
                "b": np.ascontiguousarray(b[sl]),
            }
        )
    res = bass_utils.run_bass_kernel_spmd(
        nc, in_maps, core_ids=list(range(N_CORES)), **spmd_kwargs
    )
    total = 0.0
    for r in res.results:
        total += np.asarray(r["out"])[:, :8].astype(np.float64).sum()
    mse = np.float32(total / (FULL_BATCH * N * N))
    return np.asarray(mse, dtype=np.float32), res


def kernel(**inputs) -> np.ndarray:
    try:
        result, _ = run(inputs)
        return result
    except Exception:
        import traceback

        traceback.print_exc()
        result, _ = _v4_run(inputs)
        return result
